# revision 12
# baseline (speedup 1.0000x reference)
"""Trainium2 Bass kernel for nn_EnergyDistributionCNN (3x3 conv -> unfold ->
softmax over patch -> weighted -> fold overlap-add), 8 NeuronCores.

Math (algebraically identical to the torch/jax reference):
    out = conv3x3(x, k)            cross-correlation, zero pad 1
    E   = exp(out)
    Z   = boxsum3x3(E padded with ONES)   (zero pads contribute exp(0)=1)
    U   = x / Z
    S   = boxsum3x3(U zero-padded)
    result = E * S

Sharding: row-block across 8 cores with a 3-row halo sliced on the host
(zero-filled at the global edges) -- no device-to-device communication.
Global boundary rows are handled uniformly by a per-row mask fused into the
exp's per-partition scale (exp(0*out)=1); boundary columns by host zero
padding plus static edge memsets.

On-core layout: rows on partitions, cols on the free dim, processed in
row-tiles (<=122 output rows) x width-halves. All vertical stencil mixing
runs on the TensorEngine via banded matrices; horizontal mixing is 3
column-shifted matmuls accumulated in PSUM. Everything on the PE uses
fp32r (full-rate moving operand, ~11-bit mantissa); x is fed to the PE by
bitcasting the f32 tile to f32r (the PE rounds internally; the resulting
~1e-3 conv error is well inside the 2e-2 gate).
exp runs on the ScalarEngine directly from conv's PSUM; 1/Z uses the DVE
fast reciprocal (~18 bits). Band row-mappings put every compute op at
partition base 0; the valid output rows sit at partitions [2, R+2), which
the (partition-unrestricted) output DMA reads.
"""

from contextlib import ExitStack

import numpy as np

import concourse.bacc as bacc
import concourse.mybir as mybir
import concourse.tile as tile
from concourse._compat import with_exitstack
from concourse.bass_utils import run_bass_kernel_spmd

F32 = mybir.dt.float32
F32R = mybir.dt.float32r

H = 4096
W = 4096
N_CORES = 8
RC = H // N_CORES  # rows per core
HALO = 3
RT = 122   # output rows per row-tile (RT + 6 <= 128 partitions)
WS = 2     # width splits (SBUF capacity)
WH = W // WS
C = 512    # matmul column chunk = one fp32 PSUM bank
NBUFS = 3
PS_BUFS = 3


# ---------------------------------------------------------------- host side

def _make_bands(k: np.ndarray) -> np.ndarray:
    """bands[v][p, m] = k[p-m, v] (conv, v=0..2); bands[3] = BB ones with
    p-m in 0..2 (S matmul); bands[4] = BT ones with m-p in 0..2 (Z).
    bands[5..9]: same five patterns as 4x block-diagonal 32x32 blocks, for
    the column-folded last row-tile."""
    bands = np.zeros((10, 128, 128), np.float32)
    idx = np.arange(128)
    for d in range(3):
        p = idx[d:]
        m = idx[: 128 - d]
        for v in range(3):
            bands[v, p, m] = k[d, v]
        bands[3, p, m] = 1.0
        bands[4, m, p] = 1.0
    for i in range(5):
        blk = bands[i][:32, :32]
        for b in range(4):
            bands[5 + i][32 * b : 32 * b + 32, 32 * b : 32 * b + 32] = blk
    return bands


def _make_core_inputs(x: np.ndarray, bands: np.ndarray, core: int):
    r0 = core * RC
    lo, hi = r0 - HALO, r0 + RC + HALO
    # 26 extra zero rows let the folded last tile load full 32-row blocks
    xh = np.zeros((RC + 2 * HALO + 26, W + 2 * HALO), np.float32)
    s_lo, s_hi = max(lo, 0), min(hi, H)
    xh[s_lo - lo : s_hi - lo, HALO : HALO + W] = x[s_lo:s_hi]
    gl = np.arange(lo, hi)
    mask = ((gl >= 0) & (gl < H)).astype(np.float32)[:, None]
    return {"xh": xh, "mask": mask, "bands": bands}


def _make_tiles():
    tiles = []
    o = 0
    while o < RC:
        R = min(RT, RC - o)
        tiles.append((o, R))
        o += R
    return tiles


def _chunks(total: int):
    out = []
    s = 0
    while s < total:
        out.append((s, min(C, total - s)))
        s += C
    return out


def _zchunks(total: int):
    """1024-col chunks (2 PSUM banks) with a small tail."""
    out = []
    s = 0
    while s < total:
        out.append((s, min(2 * C, total - s)))
        s += 2 * C
    return out


def _hchunks(total: int):
    """Split into ceil-even parts of ~1024 (SBUF ops, no bank limit)."""
    n = max(1, (total + C) // (2 * C))
    base, rem = divmod(total, n)
    out = []
    s = 0
    for i in range(n):
        cl = base + (1 if i < rem else 0)
        out.append((s, cl))
        s += cl
    return out


# -------------------------------------------------------------- device side

@with_exitstack
def _energy_body(ctx: ExitStack, tc, out_d, xh_d, mask_d, bands_d):
    nc = tc.nc
    Exp = mybir.ActivationFunctionType.Exp

    # ---- constants: ONE DMA for all band matrices, hi/lo split on device;
    # the folded set is materialized first (the first emitted unit needs it)
    consts = ctx.enter_context(tc.tile_pool(name="consts", bufs=1))
    bigb = consts.tile([128, 10 * 128], F32, name="bigb")
    nc.sync.dma_start(
        out=bigb.rearrange("p (i m) -> p i m", i=10),
        in_=bands_d.rearrange("i p m -> p i m"),
    )

    def load_bands(base, suffix):
        mhi = []
        for v in range(3):
            mf = bigb[:, (base + v) * 128 : (base + v + 1) * 128]
            hi = consts.tile([128, 128], F32R, name=f"mhi{suffix}{v}")
            nc.vector.tensor_copy(out=hi, in_=mf)
            mhi.append(hi)
        bb = consts.tile([128, 128], F32R, name=f"bb{suffix}")
        nc.vector.tensor_copy(out=bb, in_=bigb[:, (base + 3) * 128 : (base + 4) * 128])
        bt = consts.tile([128, 128], F32R, name=f"bt{suffix}")
        nc.vector.tensor_copy(out=bt, in_=bigb[:, (base + 4) * 128 : (base + 5) * 128])
        return mhi, bb, bt

    MhiF, BBF, BTF = load_bands(5, "f")
    Mhi, BB, BT = load_bands(0, "")
    SEGW = WH // 4

    xpool = ctx.enter_context(tc.tile_pool(name="xp", bufs=NBUFS))
    epool = ctx.enter_context(tc.tile_pool(name="ep", bufs=NBUFS))
    upool = ctx.enter_context(tc.tile_pool(name="up", bufs=NBUFS))
    apool = ctx.enter_context(tc.tile_pool(name="ap", bufs=2))
    hupool = ctx.enter_context(tc.tile_pool(name="hup", bufs=NBUFS))
    rzpool = ctx.enter_context(tc.tile_pool(name="rzp", bufs=3))
    scpool = ctx.enter_context(tc.tile_pool(name="scp", bufs=3))
    respool = ctx.enter_context(tc.tile_pool(name="resp", bufs=2))
    mpool = ctx.enter_context(tc.tile_pool(name="mp", bufs=2))
    ps_conv = ctx.enter_context(tc.tile_pool(name="psc", bufs=2, space="PSUM"))
    ps_z = ctx.enter_context(tc.tile_pool(name="psz", bufs=2, space="PSUM"))
    ps_s = ctx.enter_context(tc.tile_pool(name="pss", bufs=2, space="PSUM"))

    tiles = _make_tiles()
    Add = mybir.AluOpType.add
    Mult = mybir.AluOpType.mult
    Copy = mybir.ActivationFunctionType.Copy
    flip = [0]

    def pipeline(X, mk, Mh, bb, bt, PW, rX, rE, rS, e_edge, u_edge, emit_out):
        """Stencil chain on one (row-tile, width-half) unit.

        Engine split per stage (costs per full pass over the core's data):
          conv: 3 shifted fp32r band matmuls (PE) -> exp on Act from PSUM
          Z:    3 shifted BT matmuls (PE) -> 1/Z on DVE -> U = x*Rz (DVE stt 2x)
          H_U:  horizontal 3-tap of U: level-1 on GpSimd, level-2 on DVE
          S:    1 BB matmul on H_U (PE) -> Act copies PSUM->SBUF
          res:  E*S via scalar_tensor_tensor, alternating DVE / GpSimd
        """
        Xf = X.bitcast(F32)

        # conv + exp -> E[m, e] ~ (row base+m, col colbase+e)
        E = epool.tile([128, WH + 4], F32R, tag="E")
        for cs, cl in _chunks(PW + 4):
            pc = ps_conv.tile([128, C], F32, tag="pc")
            for v in range(3):
                nc.tensor.matmul(
                    pc[:rE, :cl],
                    Mh[v][:rX, :rE],
                    X[:rX, cs + v : cs + v + cl],
                    start=(v == 0),
                    stop=(v == 2),
                )
            nc.scalar.activation(E[:rE, cs : cs + cl], pc[:rE, :cl], Exp, scale=mk[:rE])
        e_edge(E)

        # Z (vertical+horizontal via 3 shifted BT matmuls) -> Rz -> U = x*Rz
        U = upool.tile([128, WH + 2], F32, tag="U")
        for cs, cl in _zchunks(PW + 2):
            pz = ps_z.tile([128, 2 * C], F32, tag="pz")
            for qs in range(0, cl, C):
                ql = min(C, cl - qs)
                for v in range(3):
                    nc.tensor.matmul(
                        pz[:rE, qs : qs + ql],
                        bt[:rE, :rE],
                        E[:rE, cs + qs + v : cs + qs + v + ql],
                        start=(v == 0),
                        stop=(v == 2),
                    )
            Rz = rzpool.tile([128, 2 * C], F32, tag="Rz")
            nc.vector.reciprocal_approx_fast(out=Rz[:rE, :cl], in_=pz[:rE, :cl])
            nc.vector.scalar_tensor_tensor(
                out=U[:rE, cs : cs + cl],
                in0=Xf[:rE, cs + 2 : cs + 2 + cl],
                scalar=0.0,
                in1=Rz[:rE, :cl],
                op0=Add,
                op1=Mult,
            )
        u_edge(U)

        # horizontal 3-tap of U: A = U0+U1 (GpSimd), Hu = A+U2 (DVE)
        A = apool.tile([128, WH + 1], F32, tag="A")
        for cs, cl in _hchunks(PW + 1):
            nc.gpsimd.tensor_add(
                out=A[:rE, cs : cs + cl],
                in0=U[:rE, cs : cs + cl],
                in1=U[:rE, cs + 1 : cs + 1 + cl],
            )
        Hu = hupool.tile([128, WH], F32R, tag="Hu")
        for cs, cl in _hchunks(PW):
            nc.vector.scalar_tensor_tensor(
                out=Hu[:rE, cs : cs + cl],
                in0=A[:rE, cs : cs + cl],
                scalar=0.0,
                in1=U[:rE, cs + 2 : cs + 2 + cl],
                op0=Add,
                op1=Add,
            )

        # S = BB @ Hu (1 matmul), Act copies PSUM->SBUF, res = E*S
        res = respool.tile([128, WH], F32, tag="res")
        for cs, cl in _hchunks(PW):
            Sc = scpool.tile([128, 2 * C], F32, tag="Sc")
            for qs in range(0, cl, C):
                ql = min(C, cl - qs)
                ps = ps_s.tile([128, C], F32, tag="ps")
                nc.tensor.matmul(
                    ps[:rS, :ql],
                    bb[:rE, :rS],
                    Hu[:rE, cs + qs : cs + qs + ql],
                    start=True,
                    stop=True,
                )
                nc.scalar.activation(Sc[:rS, qs : qs + ql], ps[:rS, :ql], Copy)
            if flip[0] % 3 == 2:  # every 3rd res-mul on GpSimd (load balance)
                nc.gpsimd.tensor_mul(
                    out=res[:rS, cs : cs + cl],
                    in0=E[:rS, cs + 2 : cs + 2 + cl],
                    in1=Sc[:rS, :cl],
                )
            else:
                nc.vector.scalar_tensor_tensor(
                    out=res[:rS, cs : cs + cl],
                    in0=E[:rS, cs + 2 : cs + 2 + cl],
                    scalar=0.0,
                    in1=Sc[:rS, :cl],
                    op0=Add,
                    op1=Mult,
                )
            flip[0] += 1
        emit_out(res)

    def fold_unit(o, R, h):
        # Column-folded last row-tile: 4 width-segments of one half stacked
        # on 32-partition blocks, block-diagonal bands, ops span all 128
        # partitions (off-band lanes hold finite junk; masked exp gives
        # E=1 and the extended Z band keeps Z>0 there).
        g0 = h * WH
        mk = mpool.tile([128, 1], F32, tag="mk")
        nc.vector.memset(mk, 0.0)
        for b in range(4):
            nc.sync.dma_start(
                out=mk[32 * b : 32 * b + R + 4], in_=mask_d[o + 1 : o + R + 5, :]
            )
        X = xpool.tile([128, WH + 6], F32R, tag="X")
        for b in range(4):
            nc.sync.dma_start(
                out=X[32 * b : 32 * b + 32, : SEGW + 6],
                in_=xh_d[
                    o : o + 32, g0 + b * SEGW : g0 + b * SEGW + SEGW + 6
                ].bitcast(F32R),
            )

        def e_edge(E):
            if h == 0:
                nc.vector.memset(E[0:32, 0:2].bitcast(F32), 1.0)
            if h == WS - 1:
                nc.vector.memset(E[96:128, SEGW + 2 : SEGW + 4].bitcast(F32), 1.0)

        def u_edge(U):
            if h == 0:
                nc.vector.memset(U[0:32, 0:1], 0.0)
            if h == WS - 1:
                nc.vector.memset(U[96:128, SEGW + 1 : SEGW + 2], 0.0)

        def emit_out(res):
            for b in range(4):
                nc.sync.dma_start(
                    out=out_d[o : o + R, g0 + b * SEGW : g0 + (b + 1) * SEGW],
                    in_=res[32 * b + 2 : 32 * b + 2 + R, :SEGW],
                )

        pipeline(X, mk, MhiF, BBF, BTF, SEGW, 128, 128, 128, e_edge, u_edge, emit_out)

    def normal_tile(o, R):
        mk = mpool.tile([128, 1], F32, tag="mk")
        nc.sync.dma_start(out=mk[: R + 4], in_=mask_d[o + 1 : o + R + 5, :])
        for h in range(WS):
            g0 = h * WH
            # X[p, j] <-> (row r-3+p, global col g0-3+j)
            X = xpool.tile([128, WH + 6], F32R, tag="X")
            nc.sync.dma_start(
                out=X[: R + 6, :],
                in_=xh_d[o : o + R + 6, g0 : g0 + WH + 6].bitcast(F32R),
            )

            def e_edge(E, R=R, h=h):
                if h == 0:
                    nc.vector.memset(E[: R + 4, 0:2].bitcast(F32), 1.0)
                if h == WS - 1:
                    nc.vector.memset(E[: R + 4, WH + 2 : WH + 4].bitcast(F32), 1.0)

            def u_edge(U, R=R, h=h):
                if h == 0:
                    nc.vector.memset(U[: R + 4, 0:1], 0.0)
                if h == WS - 1:
                    nc.vector.memset(U[: R + 4, WH + 1 : WH + 2], 0.0)

            def emit_out(res, o=o, R=R, g0=g0):
                nc.sync.dma_start(
                    out=out_d[o : o + R, g0 : g0 + WH], in_=res[2 : R + 2, :WH]
                )

            pipeline(
                X, mk, Mhi, BB, BT, WH, R + 6, R + 4, R + 2, e_edge, u_edge, emit_out
            )

    of, Rf = tiles[-1]
    if len(tiles) > 1 and Rf <= 26:
        # cheap folded units at both pipeline edges: fast fill and drain
        fold_unit(of, Rf, 0)
        for o, R in tiles[:-1]:
            normal_tile(o, R)
        fold_unit(of, Rf, WS - 1)
    else:
        for o, R in tiles:
            normal_tile(o, R)


_CACHE: dict = {}


def _build():
    if "nc" in _CACHE:
        return _CACHE["nc"]
    nc = bacc.Bacc(
        "TRN2", target_bir_lowering=False, debug=False, num_devices=N_CORES
    )
    xh_d = nc.dram_tensor(
        "xh", (RC + 2 * HALO + 26, W + 2 * HALO), F32, kind="ExternalInput"
    ).ap()
    mask_d = nc.dram_tensor("mask", (RC + 2 * HALO, 1), F32, kind="ExternalInput").ap()
    bands_d = nc.dram_tensor("bands", (10, 128, 128), F32, kind="ExternalInput").ap()
    out_d = nc.dram_tensor("out", (RC, W), F32, kind="ExternalOutput").ap()
    with tile.TileContext(nc) as tc:
        _energy_body(tc, out_d, xh_d, mask_d, bands_d)
    nc.compile()
    _CACHE["nc"] = nc
    return nc


def kernel(shareable_energy: np.ndarray, kernel: np.ndarray, **_run_kw) -> np.ndarray:
    x = np.ascontiguousarray(np.asarray(shareable_energy, np.float32))
    k = np.asarray(kernel, np.float32)
    assert x.shape == (H, W), x.shape
    nc = _build()
    bands = _make_bands(k)
    in_maps = [_make_core_inputs(x, bands, core) for core in range(N_CORES)]
    r = run_bass_kernel_spmd(nc, in_maps, core_ids=list(range(N_CORES)), **_run_kw)
    out = np.concatenate([res["out"] for res in r.results], axis=0)
    if _run_kw:
        _CACHE["last_result"] = r
    return out



# revision 13
# speedup vs baseline: 1.3322x; 1.3322x over previous
"""Trainium2 Bass kernel for nn_EnergyDistributionCNN (3x3 conv -> unfold ->
softmax over patch -> weighted -> fold overlap-add), 8 NeuronCores.

Math (algebraically identical to the torch/jax reference):
    out = conv3x3(x, k)            cross-correlation, zero pad 1
    E   = exp(out)
    Z   = boxsum3x3(E padded with ONES)   (zero pads contribute exp(0)=1)
    U   = x / Z
    S   = boxsum3x3(U zero-padded)
    result = E * S

Sharding: row-block across 8 cores with a 3-row halo sliced on the host
(zero-filled at the global edges) -- no device-to-device communication.
Global boundary rows are handled uniformly by a per-row mask fused into the
exp's per-partition scale (exp(0*out)=1); boundary columns by host zero
padding plus static edge memsets.

On-core layout: rows on partitions, cols on the free dim, processed in
row-tiles (<=122 output rows) x width-halves. All vertical stencil mixing
runs on the TensorEngine via banded matrices; horizontal mixing is 3
column-shifted matmuls accumulated in PSUM. Everything on the PE uses
fp32r (full-rate moving operand, ~11-bit mantissa); x is fed to the PE by
bitcasting the f32 tile to f32r (the PE rounds internally; the resulting
~1e-3 conv error is well inside the 2e-2 gate).
exp runs on the ScalarEngine directly from conv's PSUM; 1/Z uses the DVE
fast reciprocal (~18 bits). Band row-mappings put every compute op at
partition base 0; the valid output rows sit at partitions [2, R+2), which
the (partition-unrestricted) output DMA reads.
"""

from contextlib import ExitStack

import numpy as np

import concourse.bacc as bacc
import concourse.mybir as mybir
import concourse.tile as tile
from concourse._compat import with_exitstack
from concourse.bass_utils import run_bass_kernel_spmd

F32 = mybir.dt.float32
F32R = mybir.dt.float32r

H = 4096
W = 4096
N_CORES = 8
RC = H // N_CORES  # rows per core
HALO = 3
RT = 122   # output rows per row-tile (RT + 6 <= 128 partitions)
WS = 2     # width splits (SBUF capacity)
WH = W // WS
C = 512    # matmul column chunk = one fp32 PSUM bank
NBUFS = 3
PS_BUFS = 3


# ---------------------------------------------------------------- host side

def _make_bands(k: np.ndarray) -> np.ndarray:
    """bands[v][p, m] = k[p-m, v] (conv, v=0..2); bands[3] = BB ones with
    p-m in 0..2 (S matmul); bands[4] = BT ones with m-p in 0..2 (Z).
    bands[5..9]: same five patterns as 4x block-diagonal 32x32 blocks, for
    the column-folded last row-tile."""
    bands = np.zeros((10, 128, 128), np.float32)
    idx = np.arange(128)
    for d in range(3):
        p = idx[d:]
        m = idx[: 128 - d]
        for v in range(3):
            bands[v, p, m] = k[d, v]
        bands[3, p, m] = 1.0
        bands[4, m, p] = 1.0
    for i in range(5):
        blk = bands[i][:32, :32]
        for b in range(4):
            bands[5 + i][32 * b : 32 * b + 32, 32 * b : 32 * b + 32] = blk
    return bands


def _make_core_inputs(x: np.ndarray, bands: np.ndarray, core: int):
    r0 = core * RC
    lo, hi = r0 - HALO, r0 + RC + HALO
    # 26 extra zero rows let the folded last tile load full 32-row blocks
    xh = np.zeros((RC + 2 * HALO + 26, W + 2 * HALO), np.float32)
    s_lo, s_hi = max(lo, 0), min(hi, H)
    xh[s_lo - lo : s_hi - lo, HALO : HALO + W] = x[s_lo:s_hi]
    gl = np.arange(lo, hi)
    mask = ((gl >= 0) & (gl < H)).astype(np.float32)[:, None]
    return {"xh": xh, "mask": mask, "bands": bands}


def _make_tiles():
    tiles = []
    o = 0
    while o < RC:
        R = min(RT, RC - o)
        tiles.append((o, R))
        o += R
    return tiles


def _chunks(total: int):
    out = []
    s = 0
    while s < total:
        out.append((s, min(C, total - s)))
        s += C
    return out


def _zchunks(total: int):
    """1024-col chunks (2 PSUM banks) with a small tail."""
    out = []
    s = 0
    while s < total:
        out.append((s, min(2 * C, total - s)))
        s += 2 * C
    return out


def _hchunks(total: int):
    """Split into ceil-even parts of ~1024 (SBUF ops, no bank limit)."""
    n = max(1, (total + C) // (2 * C))
    base, rem = divmod(total, n)
    out = []
    s = 0
    for i in range(n):
        cl = base + (1 if i < rem else 0)
        out.append((s, cl))
        s += cl
    return out


# -------------------------------------------------------------- device side

@with_exitstack
def _energy_body(ctx: ExitStack, tc, out_d, xh_d, mask_d, bands_d):
    nc = tc.nc
    Exp = mybir.ActivationFunctionType.Exp

    # ---- constants: ONE DMA for all band matrices, hi/lo split on device;
    # the folded set is materialized first (the first emitted unit needs it)
    consts = ctx.enter_context(tc.tile_pool(name="consts", bufs=1))
    bigb = consts.tile([128, 10 * 128], F32, name="bigb")
    nc.sync.dma_start(
        out=bigb.rearrange("p (i m) -> p i m", i=10),
        in_=bands_d.rearrange("i p m -> p i m"),
    )

    def load_bands(base, suffix):
        mhi = []
        for v in range(3):
            mf = bigb[:, (base + v) * 128 : (base + v + 1) * 128]
            hi = consts.tile([128, 128], F32R, name=f"mhi{suffix}{v}")
            nc.vector.tensor_copy(out=hi, in_=mf)
            mhi.append(hi)
        bb = consts.tile([128, 128], F32R, name=f"bb{suffix}")
        nc.vector.tensor_copy(out=bb, in_=bigb[:, (base + 3) * 128 : (base + 4) * 128])
        bt = consts.tile([128, 128], F32R, name=f"bt{suffix}")
        nc.vector.tensor_copy(out=bt, in_=bigb[:, (base + 4) * 128 : (base + 5) * 128])
        return mhi, bb, bt

    MhiF, BBF, BTF = load_bands(5, "f")
    Mhi, BB, BT = load_bands(0, "")
    SEGW = WH // 4

    xpool = ctx.enter_context(tc.tile_pool(name="xp", bufs=3))
    epool = ctx.enter_context(tc.tile_pool(name="ep", bufs=5))
    upool = ctx.enter_context(tc.tile_pool(name="up", bufs=5))
    rzpool = ctx.enter_context(tc.tile_pool(name="rzp", bufs=3))
    respool = ctx.enter_context(tc.tile_pool(name="resp", bufs=3))
    mpool = ctx.enter_context(tc.tile_pool(name="mp", bufs=2))
    ps_conv = ctx.enter_context(tc.tile_pool(name="psc", bufs=3, space="PSUM"))
    ps_z = ctx.enter_context(tc.tile_pool(name="psz", bufs=2, space="PSUM"))
    ps_s = ctx.enter_context(tc.tile_pool(name="pss", bufs=3, space="PSUM"))

    tiles = _make_tiles()

    class Unit:
        """One (row-tile, width-half) pipeline unit, emitted in 3 phases so
        every engine's in-order queue only ever waits on work from >=1 units
        earlier (no head-of-line blocking on the bottleneck PE):
          p1: X/mask DMA; conv (3 shifted fp32r band matmuls, PE) -> exp (Act)
          p2: Z (3 shifted BT matmuls, PE) -> 1/Z (DVE) -> U = x*Rz (GpSimd)
          p3: S (3 shifted BB matmuls on U, PE) -> res = E*S (DVE, PSUM in)
        """

        def __init__(self, kind, o, R, h):
            self.kind, self.o, self.R, self.h = kind, o, R, h
            self.Mh, self.bb, self.bt = (Mhi, BB, BT) if kind == "n" else (
                MhiF, BBF, BTF)
            self.rX, self.rE, self.rS = (
                (R + 6, R + 4, R + 2) if kind == "n" else (128, 128, 128))
            self.PW = WH if kind == "n" else SEGW
            self.g0 = h * WH

        def p1(self):
            o, R, h, g0 = self.o, self.R, self.h, self.g0
            mk = mpool.tile([128, 1], F32, tag="mk")
            if self.kind == "n":
                nc.sync.dma_start(out=mk[: R + 4], in_=mask_d[o + 1 : o + R + 5, :])
            else:
                nc.gpsimd.memset(mk, 0.0)
                for b in range(4):
                    nc.sync.dma_start(
                        out=mk[32 * b : 32 * b + R + 4],
                        in_=mask_d[o + 1 : o + R + 5, :],
                    )
            X = xpool.tile([128, WH + 6], F32R, tag="X")
            if self.kind == "n":
                nc.sync.dma_start(
                    out=X[: self.rX, :],
                    in_=xh_d[o : o + R + 6, g0 : g0 + WH + 6].bitcast(F32R),
                )
            else:
                for b in range(4):
                    nc.sync.dma_start(
                        out=X[32 * b : 32 * b + 32, : SEGW + 6],
                        in_=xh_d[
                            o : o + 32, g0 + b * SEGW : g0 + b * SEGW + SEGW + 6
                        ].bitcast(F32R),
                    )
            self.X, self.mk = X, mk

            rX, rE = self.rX, self.rE
            E = epool.tile([128, WH + 4], F32R, tag="E")
            for cs, cl in _chunks(self.PW + 4):
                pc = ps_conv.tile([128, C], F32, tag="pc")
                for v in range(3):
                    nc.tensor.matmul(
                        pc[:rE, :cl],
                        self.Mh[v][:rX, :rE],
                        X[:rX, cs + v : cs + v + cl],
                        start=(v == 0),
                        stop=(v == 2),
                    )
                nc.scalar.activation(E[:rE, cs : cs + cl], pc[:rE, :cl], Exp, scale=mk[:rE])
            # global-edge columns of E represent pad pixels: exp(0) = 1
            if self.kind == "n":
                if h == 0:
                    nc.vector.memset(E[:rE, 0:2].bitcast(F32), 1.0)
                if h == WS - 1:
                    nc.vector.memset(E[:rE, self.PW + 2 : self.PW + 4].bitcast(F32), 1.0)
            else:
                if h == 0:
                    nc.vector.memset(E[0:32, 0:2].bitcast(F32), 1.0)
                if h == WS - 1:
                    nc.vector.memset(E[96:128, SEGW + 2 : SEGW + 4].bitcast(F32), 1.0)
            self.E = E

        def p2(self):
            rE, h = self.rE, self.h
            E, X = self.E, self.X
            U = upool.tile([128, WH + 2], F32R, tag="U")
            for cs, cl in _chunks(self.PW + 2):
                pz = ps_z.tile([128, C], F32, tag="pz")
                for v in range(3):
                    nc.tensor.matmul(
                        pz[:rE, :cl],
                        self.bt[:rE, :rE],
                        E[:rE, cs + v : cs + v + cl],
                        start=(v == 0),
                        stop=(v == 2),
                    )
                Rz = rzpool.tile([128, C], F32, tag="Rz")
                nc.vector.reciprocal_approx_fast(out=Rz[:rE, :cl], in_=pz[:rE, :cl])
                nc.gpsimd.tensor_mul(
                    out=U[:rE, cs : cs + cl],
                    in0=X[:rE, cs + 2 : cs + 2 + cl],
                    in1=Rz[:rE, :cl],
                )
            # U at global-edge pad columns is 0 (fold drops OOB)
            if self.kind == "n":
                if h == 0:
                    nc.gpsimd.memset(U[:rE, 0:1].bitcast(F32), 0.0)
                if h == WS - 1:
                    nc.gpsimd.memset(U[:rE, self.PW + 1 : self.PW + 2].bitcast(F32), 0.0)
            else:
                if h == 0:
                    nc.gpsimd.memset(U[0:32, 0:1].bitcast(F32), 0.0)
                if h == WS - 1:
                    nc.gpsimd.memset(U[96:128, SEGW + 1 : SEGW + 2].bitcast(F32), 0.0)
            self.U = U

        def p3(self):
            o, R, g0 = self.o, self.R, self.g0
            rE, rS = self.rE, self.rS
            E, U = self.E, self.U
            res = respool.tile([128, WH], F32, tag="res")
            for cs, cl in _chunks(self.PW):
                ps = ps_s.tile([128, C], F32, tag="ps")
                for v in range(3):
                    nc.tensor.matmul(
                        ps[:rS, :cl],
                        self.bb[:rE, :rS],
                        U[:rE, cs + v : cs + v + cl],
                        start=(v == 0),
                        stop=(v == 2),
                    )
                nc.vector.tensor_mul(
                    out=res[:rS, cs : cs + cl],
                    in0=E[:rS, cs + 2 : cs + 2 + cl],
                    in1=ps[:rS, :cl],
                )
            if self.kind == "n":
                nc.sync.dma_start(
                    out=out_d[o : o + R, g0 : g0 + WH], in_=res[2 : R + 2, :WH]
                )
            else:
                for b in range(4):
                    nc.sync.dma_start(
                        out=out_d[o : o + R, g0 + b * SEGW : g0 + (b + 1) * SEGW],
                        in_=res[32 * b + 2 : 32 * b + 2 + R, :SEGW],
                    )

    of, Rf = tiles[-1]
    units = [Unit("f", of, Rf, 0)]
    for o, R in tiles[:-1]:
        for h in range(WS):
            units.append(Unit("n", o, R, h))
    units.append(Unit("f", of, Rf, WS - 1))

    LAG2, LAG3 = 1, 3
    n = len(units)
    for i in range(n + LAG3):
        if i < n:
            units[i].p1()
        if 0 <= i - LAG2 < n:
            units[i - LAG2].p2()
        if 0 <= i - LAG3 < n:
            units[i - LAG3].p3()


_CACHE: dict = {}


def _build():
    if "nc" in _CACHE:
        return _CACHE["nc"]
    nc = bacc.Bacc(
        "TRN2", target_bir_lowering=False, debug=False, num_devices=N_CORES
    )
    xh_d = nc.dram_tensor(
        "xh", (RC + 2 * HALO + 26, W + 2 * HALO), F32, kind="ExternalInput"
    ).ap()
    mask_d = nc.dram_tensor("mask", (RC + 2 * HALO, 1), F32, kind="ExternalInput").ap()
    bands_d = nc.dram_tensor("bands", (10, 128, 128), F32, kind="ExternalInput").ap()
    out_d = nc.dram_tensor("out", (RC, W), F32, kind="ExternalOutput").ap()
    with tile.TileContext(nc) as tc:
        _energy_body(tc, out_d, xh_d, mask_d, bands_d)
    nc.compile()
    _CACHE["nc"] = nc
    return nc


def kernel(shareable_energy: np.ndarray, kernel: np.ndarray, **_run_kw) -> np.ndarray:
    x = np.ascontiguousarray(np.asarray(shareable_energy, np.float32))
    k = np.asarray(kernel, np.float32)
    assert x.shape == (H, W), x.shape
    nc = _build()
    bands = _make_bands(k)
    in_maps = [_make_core_inputs(x, bands, core) for core in range(N_CORES)]
    r = run_bass_kernel_spmd(nc, in_maps, core_ids=list(range(N_CORES)), **_run_kw)
    out = np.concatenate([res["out"] for res in r.results], axis=0)
    if _run_kw:
        _CACHE["last_result"] = r
    return out



# revision 17
# speedup vs baseline: 1.3500x; 1.0134x over previous
"""Trainium2 Bass kernel for nn_EnergyDistributionCNN (3x3 conv -> unfold ->
softmax over patch -> weighted -> fold overlap-add), 8 NeuronCores.

Math (algebraically identical to the torch/jax reference):
    out = conv3x3(x, k)            cross-correlation, zero pad 1
    E   = exp(out)
    Z   = boxsum3x3(E padded with ONES)   (zero pads contribute exp(0)=1)
    U   = x / Z
    S   = boxsum3x3(U zero-padded)
    result = E * S

Sharding: row-block across 8 cores with a 3-row halo sliced on the host
(zero-filled at the global edges) -- no device-to-device communication.
Global boundary rows are handled uniformly by a per-row mask fused into the
exp's per-partition scale (exp(0*out)=1); boundary columns by host zero
padding plus static edge memsets.

On-core layout: rows on partitions, cols on the free dim, processed in
row-tiles (<=122 output rows) x width-halves. All vertical stencil mixing
runs on the TensorEngine via banded matrices; horizontal mixing is 3
column-shifted matmuls accumulated in PSUM. Everything on the PE uses
fp32r (full-rate moving operand, ~11-bit mantissa); x is fed to the PE by
bitcasting the f32 tile to f32r (the PE rounds internally; the resulting
~1e-3 conv error is well inside the 2e-2 gate).
exp runs on the ScalarEngine directly from conv's PSUM; 1/Z uses the DVE
fast reciprocal (~18 bits). Band row-mappings put every compute op at
partition base 0; the valid output rows sit at partitions [2, R+2), which
the (partition-unrestricted) output DMA reads.
"""

from contextlib import ExitStack

import numpy as np

import concourse.bacc as bacc
import concourse.mybir as mybir
import concourse.tile as tile
from concourse._compat import with_exitstack
from concourse.bass_utils import run_bass_kernel_spmd

F32 = mybir.dt.float32
F32R = mybir.dt.float32r

H = 4096
W = 4096
N_CORES = 8
RC = H // N_CORES  # rows per core
HALO = 3
RT = 122   # output rows per row-tile (RT + 6 <= 128 partitions)
WS = 2     # width splits (SBUF capacity)
WH = W // WS
C = 512    # matmul column chunk = one fp32 PSUM bank
NBUFS = 3
PS_BUFS = 3


# ---------------------------------------------------------------- host side

def _make_bands(k: np.ndarray) -> np.ndarray:
    """bands[v][p, m] = k[p-m, v] (conv, v=0..2); bands[3] = BB ones with
    p-m in 0..2 (S matmul); bands[4] = BT ones with m-p in 0..2 (Z).
    bands[5..9]: same five patterns as 4x block-diagonal 32x32 blocks, for
    the column-folded last row-tile."""
    bands = np.zeros((10, 128, 128), np.float32)
    idx = np.arange(128)
    for d in range(3):
        p = idx[d:]
        m = idx[: 128 - d]
        for v in range(3):
            bands[v, p, m] = k[d, v]
        bands[3, p, m] = 1.0
        bands[4, m, p] = 1.0
    for i in range(5):
        blk = bands[i][:32, :32]
        for b in range(4):
            bands[5 + i][32 * b : 32 * b + 32, 32 * b : 32 * b + 32] = blk
    return bands


def _make_core_inputs(x: np.ndarray, bands: np.ndarray, core: int):
    r0 = core * RC
    lo, hi = r0 - HALO, r0 + RC + HALO
    # 2 extra zero rows let the folded last tile load full 32-row blocks
    xh = np.zeros((RC + 2 * HALO + 2, W + 2 * HALO), np.float32)
    s_lo, s_hi = max(lo, 0), min(hi, H)
    xh[s_lo - lo : s_hi - lo, HALO : HALO + W] = x[s_lo:s_hi]
    gl = np.arange(lo, hi)
    mask = ((gl >= 0) & (gl < H)).astype(np.float32)[:, None]
    return {"xh": xh, "mask": mask, "bands": bands}


def _make_tiles():
    tiles = []
    o = 0
    while o < RC:
        R = min(RT, RC - o)
        tiles.append((o, R))
        o += R
    return tiles


def _chunks(total: int):
    out = []
    s = 0
    while s < total:
        out.append((s, min(C, total - s)))
        s += C
    return out


def _zchunks(total: int):
    """1024-col chunks (2 PSUM banks) with a small tail."""
    out = []
    s = 0
    while s < total:
        out.append((s, min(2 * C, total - s)))
        s += 2 * C
    return out


def _hchunks(total: int):
    """Split into ceil-even parts of ~1024 (SBUF ops, no bank limit)."""
    n = max(1, (total + C) // (2 * C))
    base, rem = divmod(total, n)
    out = []
    s = 0
    for i in range(n):
        cl = base + (1 if i < rem else 0)
        out.append((s, cl))
        s += cl
    return out


# -------------------------------------------------------------- device side

@with_exitstack
def _energy_body(ctx: ExitStack, tc, out_d, xh_d, mask_d, bands_d):
    nc = tc.nc
    Exp = mybir.ActivationFunctionType.Exp

    # ---- constants: band matrices DMA'd straight to f32r SBUF (byte copy);
    # the folded set is its own DMA, first -- the first emitted unit needs it
    consts = ctx.enter_context(tc.tile_pool(name="consts", bufs=1))
    bigb = consts.tile([128, 10 * 128], F32R, name="bigb")
    bands_r = bands_d.bitcast(F32R)
    nc.sync.dma_start(
        out=bigb[:, 5 * 128 :].rearrange("p (i m) -> p i m", i=5),
        in_=bands_r[5:10].rearrange("i p m -> p i m"),
    )
    nc.sync.dma_start(
        out=bigb[:, : 5 * 128].rearrange("p (i m) -> p i m", i=5),
        in_=bands_r[0:5].rearrange("i p m -> p i m"),
    )

    def band_views(base):
        mh = [bigb[:, (base + v) * 128 : (base + v + 1) * 128] for v in range(3)]
        bb = bigb[:, (base + 3) * 128 : (base + 4) * 128]
        bt = bigb[:, (base + 4) * 128 : (base + 5) * 128]
        return mh, bb, bt

    MhiF, BBF, BTF = band_views(5)
    Mhi, BB, BT = band_views(0)
    SEGW = WH // 4

    xpool = ctx.enter_context(tc.tile_pool(name="xp", bufs=3))
    epool = ctx.enter_context(tc.tile_pool(name="ep", bufs=5))
    upool = ctx.enter_context(tc.tile_pool(name="up", bufs=5))
    rzpool = ctx.enter_context(tc.tile_pool(name="rzp", bufs=3))
    respool = ctx.enter_context(tc.tile_pool(name="resp", bufs=3))
    mpool = ctx.enter_context(tc.tile_pool(name="mp", bufs=2))
    ps_conv = ctx.enter_context(tc.tile_pool(name="psc", bufs=3, space="PSUM"))
    ps_z = ctx.enter_context(tc.tile_pool(name="psz", bufs=2, space="PSUM"))
    ps_s = ctx.enter_context(tc.tile_pool(name="pss", bufs=3, space="PSUM"))

    tiles = _make_tiles()

    class Unit:
        """One (row-tile, width-half) pipeline unit, emitted in 3 phases so
        every engine's in-order queue only ever waits on work from >=1 units
        earlier (no head-of-line blocking on the bottleneck PE):
          p1: X/mask DMA; conv (3 shifted fp32r band matmuls, PE) -> exp (Act)
          p2: Z (3 shifted BT matmuls, PE) -> 1/Z (DVE) -> U = x*Rz (GpSimd)
          p3: S (3 shifted BB matmuls on U, PE) -> res = E*S (DVE, PSUM in)
        """

        def __init__(self, kind, o, R, h):
            self.kind, self.o, self.R, self.h = kind, o, R, h
            self.Mh, self.bb, self.bt = (Mhi, BB, BT) if kind == "n" else (
                MhiF, BBF, BTF)
            self.rX, self.rE, self.rS = (
                (R + 6, R + 4, R + 2) if kind == "n" else (128, 128, 128))
            self.PW = WH if kind == "n" else SEGW
            self.g0 = h * WH

        def p1(self):
            o, R, h, g0 = self.o, self.R, self.h, self.g0
            mk = mpool.tile([128, 1], F32, tag="mk")
            if self.kind == "n":
                nc.sync.dma_start(out=mk[: R + 4], in_=mask_d[o + 1 : o + R + 5, :])
            else:
                nc.gpsimd.memset(mk, 0.0)
                for b in range(4):
                    nc.sync.dma_start(
                        out=mk[32 * b : 32 * b + R + 4],
                        in_=mask_d[o + 1 : o + R + 5, :],
                    )
            X = xpool.tile([128, WH + 6], F32R, tag="X")
            if self.kind == "n":
                hw2 = (WH + 6) // 2
                nc.sync.dma_start(
                    out=X[: self.rX, :hw2],
                    in_=xh_d[o : o + R + 6, g0 : g0 + hw2].bitcast(F32R),
                )
                nc.sync.dma_start(
                    out=X[: self.rX, hw2:],
                    in_=xh_d[o : o + R + 6, g0 + hw2 : g0 + WH + 6].bitcast(F32R),
                )
            else:
                for b in range(4):
                    nc.sync.dma_start(
                        out=X[32 * b : 32 * b + 32, : SEGW + 6],
                        in_=xh_d[
                            o : o + 32, g0 + b * SEGW : g0 + b * SEGW + SEGW + 6
                        ].bitcast(F32R),
                    )
            self.X, self.mk = X, mk

            rX, rE = self.rX, self.rE
            E = epool.tile([128, WH + 4], F32R, tag="E")
            for cs, cl in _chunks(self.PW + 4):
                pc = ps_conv.tile([128, C], F32, tag="pc")
                for v in range(3):
                    nc.tensor.matmul(
                        pc[:rE, :cl],
                        self.Mh[v][:rX, :rE],
                        X[:rX, cs + v : cs + v + cl],
                        start=(v == 0),
                        stop=(v == 2),
                    )
                nc.scalar.activation(E[:rE, cs : cs + cl], pc[:rE, :cl], Exp, scale=mk[:rE])
            # global-edge columns of E represent pad pixels: exp(0) = 1
            if self.kind == "n":
                if h == 0:
                    nc.vector.memset(E[:rE, 0:2].bitcast(F32), 1.0)
                if h == WS - 1:
                    nc.vector.memset(E[:rE, self.PW + 2 : self.PW + 4].bitcast(F32), 1.0)
            else:
                if h == 0:
                    nc.vector.memset(E[0:32, 0:2].bitcast(F32), 1.0)
                if h == WS - 1:
                    nc.vector.memset(E[96:128, SEGW + 2 : SEGW + 4].bitcast(F32), 1.0)
            self.E = E

        def p2(self):
            rE, h = self.rE, self.h
            E, X = self.E, self.X
            U = upool.tile([128, WH + 2], F32R, tag="U")
            for cs, cl in _chunks(self.PW + 2):
                pz = ps_z.tile([128, C], F32, tag="pz")
                for v in range(3):
                    nc.tensor.matmul(
                        pz[:rE, :cl],
                        self.bt[:rE, :rE],
                        E[:rE, cs + v : cs + v + cl],
                        start=(v == 0),
                        stop=(v == 2),
                    )
                Rz = rzpool.tile([128, C], F32, tag="Rz")
                nc.vector.reciprocal_approx_fast(out=Rz[:rE, :cl], in_=pz[:rE, :cl])
                nc.gpsimd.tensor_mul(
                    out=U[:rE, cs : cs + cl],
                    in0=X[:rE, cs + 2 : cs + 2 + cl],
                    in1=Rz[:rE, :cl],
                )
            # U at global-edge pad columns is 0 (fold drops OOB)
            if self.kind == "n":
                if h == 0:
                    nc.gpsimd.memset(U[:rE, 0:1].bitcast(F32), 0.0)
                if h == WS - 1:
                    nc.gpsimd.memset(U[:rE, self.PW + 1 : self.PW + 2].bitcast(F32), 0.0)
            else:
                if h == 0:
                    nc.gpsimd.memset(U[0:32, 0:1].bitcast(F32), 0.0)
                if h == WS - 1:
                    nc.gpsimd.memset(U[96:128, SEGW + 1 : SEGW + 2].bitcast(F32), 0.0)
            self.U = U

        def p3(self):
            o, R, g0 = self.o, self.R, self.g0
            rE, rS = self.rE, self.rS
            E, U = self.E, self.U
            res = respool.tile([128, WH], F32, tag="res")
            for cs, cl in _chunks(self.PW):
                ps = ps_s.tile([128, C], F32, tag="ps")
                for v in range(3):
                    nc.tensor.matmul(
                        ps[:rS, :cl],
                        self.bb[:rE, :rS],
                        U[:rE, cs + v : cs + v + cl],
                        start=(v == 0),
                        stop=(v == 2),
                    )
                nc.vector.tensor_mul(
                    out=res[:rS, cs : cs + cl],
                    in0=E[:rS, cs + 2 : cs + 2 + cl],
                    in1=ps[:rS, :cl],
                )
            if self.kind == "n":
                nc.sync.dma_start(
                    out=out_d[o : o + R, g0 : g0 + WH], in_=res[2 : R + 2, :WH]
                )
            else:
                for b in range(4):
                    nc.sync.dma_start(
                        out=out_d[o : o + R, g0 + b * SEGW : g0 + (b + 1) * SEGW],
                        in_=res[32 * b + 2 : 32 * b + 2 + R, :SEGW],
                    )

    of, Rf = tiles[-1]
    units = [Unit("f", of, Rf, 0)]
    for o, R in tiles[:-1]:
        for h in range(WS):
            units.append(Unit("n", o, R, h))
    units.append(Unit("f", of, Rf, WS - 1))

    LAG2, LAG3 = 1, 2
    n = len(units)
    for i in range(n + LAG3):
        if i < n:
            units[i].p1()
        if 0 <= i - LAG2 < n:
            units[i - LAG2].p2()
        if 0 <= i - LAG3 < n:
            units[i - LAG3].p3()


_CACHE: dict = {}


def _build():
    if "nc" in _CACHE:
        return _CACHE["nc"]
    nc = bacc.Bacc(
        "TRN2", target_bir_lowering=False, debug=False, num_devices=N_CORES
    )
    xh_d = nc.dram_tensor(
        "xh", (RC + 2 * HALO + 2, W + 2 * HALO), F32, kind="ExternalInput"
    ).ap()
    mask_d = nc.dram_tensor("mask", (RC + 2 * HALO, 1), F32, kind="ExternalInput").ap()
    bands_d = nc.dram_tensor("bands", (10, 128, 128), F32, kind="ExternalInput").ap()
    out_d = nc.dram_tensor("out", (RC, W), F32, kind="ExternalOutput").ap()
    with tile.TileContext(nc) as tc:
        _energy_body(tc, out_d, xh_d, mask_d, bands_d)
    nc.compile()
    _CACHE["nc"] = nc
    return nc


def kernel(shareable_energy: np.ndarray, kernel: np.ndarray, **_run_kw) -> np.ndarray:
    x = np.ascontiguousarray(np.asarray(shareable_energy, np.float32))
    k = np.asarray(kernel, np.float32)
    assert x.shape == (H, W), x.shape
    nc = _build()
    bands = _make_bands(k)
    in_maps = [_make_core_inputs(x, bands, core) for core in range(N_CORES)]
    r = run_bass_kernel_spmd(nc, in_maps, core_ids=list(range(N_CORES)), **_run_kw)
    out = np.concatenate([res["out"] for res in r.results], axis=0)
    if _run_kw:
        _CACHE["last_result"] = r
    return out



# revision 18
# speedup vs baseline: 1.3616x; 1.0086x over previous
"""Trainium2 Bass kernel for nn_EnergyDistributionCNN (3x3 conv -> unfold ->
softmax over patch -> weighted -> fold overlap-add), 8 NeuronCores.

Math (algebraically identical to the torch/jax reference):
    out = conv3x3(x, k)            cross-correlation, zero pad 1
    E   = exp(out)
    Z   = boxsum3x3(E padded with ONES)   (zero pads contribute exp(0)=1)
    U   = x / Z
    S   = boxsum3x3(U zero-padded)
    result = E * S

Sharding: row-block across 8 cores with a 3-row halo sliced on the host
(zero-filled at the global edges) -- no device-to-device communication.
Global boundary rows are handled uniformly by a per-row mask fused into the
exp's per-partition scale (exp(0*out)=1); boundary columns by host zero
padding plus static edge memsets.

On-core layout: rows on partitions, cols on the free dim, processed in
row-tiles (<=122 output rows) x width-halves. All vertical stencil mixing
runs on the TensorEngine via banded matrices; horizontal mixing is 3
column-shifted matmuls accumulated in PSUM. Everything on the PE uses
fp32r (full-rate moving operand, ~11-bit mantissa); x is fed to the PE by
bitcasting the f32 tile to f32r (the PE rounds internally; the resulting
~1e-3 conv error is well inside the 2e-2 gate).
exp runs on the ScalarEngine directly from conv's PSUM; 1/Z uses the DVE
fast reciprocal (~18 bits). Band row-mappings put every compute op at
partition base 0; the valid output rows sit at partitions [2, R+2), which
the (partition-unrestricted) output DMA reads.
"""

from contextlib import ExitStack

import numpy as np

import concourse.bacc as bacc
import concourse.mybir as mybir
import concourse.tile as tile
from concourse._compat import with_exitstack
from concourse.bass_utils import run_bass_kernel_spmd

F32 = mybir.dt.float32
F32R = mybir.dt.float32r

H = 4096
W = 4096
N_CORES = 8
RC = H // N_CORES  # rows per core
HALO = 3
RT = 122   # output rows per row-tile (RT + 6 <= 128 partitions)
WS = 2     # width splits (SBUF capacity)
WH = W // WS
C = 512    # matmul column chunk = one fp32 PSUM bank
NBUFS = 3
PS_BUFS = 3


# ---------------------------------------------------------------- host side

def _make_bands(k: np.ndarray) -> np.ndarray:
    """bands[v][p, m] = k[p-m, v] (conv, v=0..2); bands[3] = BB ones with
    p-m in 0..2 (S matmul); bands[4] = BT ones with m-p in 0..2 (Z).
    bands[5..9]: same five patterns as 4x block-diagonal 32x32 blocks, for
    the column-folded last row-tile."""
    bands = np.zeros((10, 128, 128), np.float32)
    idx = np.arange(128)
    for d in range(3):
        p = idx[d:]
        m = idx[: 128 - d]
        for v in range(3):
            bands[v, p, m] = k[d, v]
        bands[3, p, m] = 1.0
        bands[4, m, p] = 1.0
    for i in range(5):
        blk = bands[i][:32, :32]
        for b in range(4):
            bands[5 + i][32 * b : 32 * b + 32, 32 * b : 32 * b + 32] = blk
    return bands


def _make_core_inputs(x: np.ndarray, bands: np.ndarray, core: int):
    r0 = core * RC
    lo, hi = r0 - HALO, r0 + RC + HALO
    # 2 extra zero rows let the folded last tile load full 32-row blocks
    xh = np.zeros((RC + 2 * HALO + 2, W + 2 * HALO), np.float32)
    s_lo, s_hi = max(lo, 0), min(hi, H)
    xh[s_lo - lo : s_hi - lo, HALO : HALO + W] = x[s_lo:s_hi]
    gl = np.arange(lo, hi)
    mask = ((gl >= 0) & (gl < H)).astype(np.float32)[:, None]
    # folded-tile mask: 4 stacked 32-row blocks, rows [of+1, of+Rf+5) each
    of = (RC // RT) * RT
    Rf = RC - of
    maskf = np.zeros((128, 1), np.float32)
    for b in range(4):
        maskf[32 * b : 32 * b + Rf + 4] = mask[of + 1 : of + Rf + 5]
    return {"xh": xh, "mask": mask, "maskf": maskf, "bands": bands}


def _make_tiles():
    tiles = []
    o = 0
    while o < RC:
        R = min(RT, RC - o)
        tiles.append((o, R))
        o += R
    return tiles


def _chunks(total: int):
    out = []
    s = 0
    while s < total:
        out.append((s, min(C, total - s)))
        s += C
    return out


def _zchunks(total: int):
    """1024-col chunks (2 PSUM banks) with a small tail."""
    out = []
    s = 0
    while s < total:
        out.append((s, min(2 * C, total - s)))
        s += 2 * C
    return out


def _hchunks(total: int):
    """Split into ceil-even parts of ~1024 (SBUF ops, no bank limit)."""
    n = max(1, (total + C) // (2 * C))
    base, rem = divmod(total, n)
    out = []
    s = 0
    for i in range(n):
        cl = base + (1 if i < rem else 0)
        out.append((s, cl))
        s += cl
    return out


# -------------------------------------------------------------- device side

@with_exitstack
def _energy_body(ctx: ExitStack, tc, out_d, xh_d, mask_d, maskf_d, bands_d):
    nc = tc.nc
    Exp = mybir.ActivationFunctionType.Exp

    # ---- constants: band matrices DMA'd straight to f32r SBUF (byte copy);
    # the folded set is its own DMA, first -- the first emitted unit needs it
    consts = ctx.enter_context(tc.tile_pool(name="consts", bufs=1))
    bigb = consts.tile([128, 10 * 128], F32R, name="bigb")
    bands_r = bands_d.bitcast(F32R)
    nc.sync.dma_start(
        out=bigb[:, : 5 * 128].rearrange("p (i m) -> p i m", i=5),
        in_=bands_r[0:5].rearrange("i p m -> p i m"),
    )
    nc.sync.dma_start(
        out=bigb[:, 5 * 128 :].rearrange("p (i m) -> p i m", i=5),
        in_=bands_r[5:10].rearrange("i p m -> p i m"),
    )

    def band_views(base):
        mh = [bigb[:, (base + v) * 128 : (base + v + 1) * 128] for v in range(3)]
        bb = bigb[:, (base + 3) * 128 : (base + 4) * 128]
        bt = bigb[:, (base + 4) * 128 : (base + 5) * 128]
        return mh, bb, bt

    MhiF, BBF, BTF = band_views(5)
    Mhi, BB, BT = band_views(0)
    SEGW = WH // 4

    xpool = ctx.enter_context(tc.tile_pool(name="xp", bufs=3))
    epool = ctx.enter_context(tc.tile_pool(name="ep", bufs=5))
    upool = ctx.enter_context(tc.tile_pool(name="up", bufs=5))
    rzpool = ctx.enter_context(tc.tile_pool(name="rzp", bufs=3))
    apool = ctx.enter_context(tc.tile_pool(name="ap", bufs=2))
    respool = ctx.enter_context(tc.tile_pool(name="resp", bufs=3))
    mpool = ctx.enter_context(tc.tile_pool(name="mp", bufs=2))
    ps_conv = ctx.enter_context(tc.tile_pool(name="psc", bufs=3, space="PSUM"))
    ps_z = ctx.enter_context(tc.tile_pool(name="psz", bufs=2, space="PSUM"))
    ps_s = ctx.enter_context(tc.tile_pool(name="pss", bufs=3, space="PSUM"))

    tiles = _make_tiles()

    class Unit:
        """One (row-tile, width-half) pipeline unit, emitted in 3 phases so
        every engine's in-order queue only ever waits on work from >=1 units
        earlier (no head-of-line blocking on the bottleneck PE):
          p1: X/mask DMA; conv (3 shifted fp32r band matmuls, PE) -> exp (Act)
          p2: Z (3 shifted BT matmuls, PE) -> 1/Z (DVE) -> U = x*Rz (GpSimd)
          p3: S (3 shifted BB matmuls on U, PE) -> res = E*S (DVE, PSUM in)
        """

        def __init__(self, kind, o, R, h):
            self.kind, self.o, self.R, self.h = kind, o, R, h
            self.Mh, self.bb, self.bt = (Mhi, BB, BT) if kind == "n" else (
                MhiF, BBF, BTF)
            self.rX, self.rE, self.rS = (
                (R + 6, R + 4, R + 2) if kind == "n" else (128, 128, 128))
            self.PW = WH if kind == "n" else SEGW
            self.g0 = h * WH

        def p1(self):
            o, R, h, g0 = self.o, self.R, self.h, self.g0
            mk = mpool.tile([128, 1], F32, tag="mk")
            if self.kind == "n":
                nc.sync.dma_start(out=mk[: R + 4], in_=mask_d[o + 1 : o + R + 5, :])
            else:
                nc.sync.dma_start(out=mk, in_=maskf_d)
            X = xpool.tile([128, WH + 6], F32R, tag="X")
            if self.kind == "n":
                hw2 = (WH + 6) // 2
                nc.sync.dma_start(
                    out=X[: self.rX, :hw2],
                    in_=xh_d[o : o + R + 6, g0 : g0 + hw2].bitcast(F32R),
                )
                nc.sync.dma_start(
                    out=X[: self.rX, hw2:],
                    in_=xh_d[o : o + R + 6, g0 + hw2 : g0 + WH + 6].bitcast(F32R),
                )
            else:
                for b in range(4):
                    nc.sync.dma_start(
                        out=X[32 * b : 32 * b + 32, : SEGW + 6],
                        in_=xh_d[
                            o : o + 32, g0 + b * SEGW : g0 + b * SEGW + SEGW + 6
                        ].bitcast(F32R),
                    )
            self.X, self.mk = X, mk

            rX, rE = self.rX, self.rE
            E = epool.tile([128, WH + 4], F32R, tag="E")
            for cs, cl in _chunks(self.PW + 4):
                pc = ps_conv.tile([128, C], F32, tag="pc")
                for v in range(3):
                    nc.tensor.matmul(
                        pc[:rE, :cl],
                        self.Mh[v][:rX, :rE],
                        X[:rX, cs + v : cs + v + cl],
                        start=(v == 0),
                        stop=(v == 2),
                    )
                nc.scalar.activation(E[:rE, cs : cs + cl], pc[:rE, :cl], Exp, scale=mk[:rE])
            # global-edge columns of E represent pad pixels: exp(0) = 1
            if self.kind == "n":
                if h == 0:
                    nc.vector.memset(E[:rE, 0:2].bitcast(F32), 1.0)
                if h == WS - 1:
                    nc.vector.memset(E[:rE, self.PW + 2 : self.PW + 4].bitcast(F32), 1.0)
            else:
                if h == 0:
                    nc.vector.memset(E[0:32, 0:2].bitcast(F32), 1.0)
                if h == WS - 1:
                    nc.vector.memset(E[96:128, SEGW + 2 : SEGW + 4].bitcast(F32), 1.0)
            self.E = E

        def p2(self):
            rE, h = self.rE, self.h
            E, X = self.E, self.X
            # A1 = E0 + E1 (horizontal pre-tap), split across DVE and GpSimd;
            # Z then needs only 2 shifted matmuls: BT@A1 + BT@E2
            A1 = apool.tile([128, WH + 3], F32R, tag="A1")
            wA = self.PW + 3
            sp = (wA * 3) // 5
            nc.vector.tensor_add(
                out=A1[:rE, :sp], in0=E[:rE, :sp], in1=E[:rE, 1 : 1 + sp]
            )
            nc.gpsimd.tensor_add(
                out=A1[:rE, sp:wA], in0=E[:rE, sp:wA], in1=E[:rE, sp + 1 : wA + 1]
            )
            U = upool.tile([128, WH + 2], F32R, tag="U")
            for cs, cl in _chunks(self.PW + 2):
                pz = ps_z.tile([128, C], F32, tag="pz")
                nc.tensor.matmul(
                    pz[:rE, :cl],
                    self.bt[:rE, :rE],
                    A1[:rE, cs : cs + cl],
                    start=True,
                    stop=False,
                )
                nc.tensor.matmul(
                    pz[:rE, :cl],
                    self.bt[:rE, :rE],
                    E[:rE, cs + 2 : cs + 2 + cl],
                    start=False,
                    stop=True,
                )
                Rz = rzpool.tile([128, C], F32, tag="Rz")
                nc.vector.reciprocal_approx_fast(out=Rz[:rE, :cl], in_=pz[:rE, :cl])
                nc.gpsimd.tensor_mul(
                    out=U[:rE, cs : cs + cl],
                    in0=X[:rE, cs + 2 : cs + 2 + cl],
                    in1=Rz[:rE, :cl],
                )
            # U at global-edge pad columns is 0 (fold drops OOB)
            if self.kind == "n":
                if h == 0:
                    nc.gpsimd.memset(U[:rE, 0:1].bitcast(F32), 0.0)
                if h == WS - 1:
                    nc.gpsimd.memset(U[:rE, self.PW + 1 : self.PW + 2].bitcast(F32), 0.0)
            else:
                if h == 0:
                    nc.gpsimd.memset(U[0:32, 0:1].bitcast(F32), 0.0)
                if h == WS - 1:
                    nc.gpsimd.memset(U[96:128, SEGW + 1 : SEGW + 2].bitcast(F32), 0.0)
            self.U = U

        def p3(self):
            o, R, g0 = self.o, self.R, self.g0
            rE, rS = self.rE, self.rS
            E, U = self.E, self.U
            res = respool.tile([128, WH], F32, tag="res")
            for cs, cl in _chunks(self.PW):
                ps = ps_s.tile([128, C], F32, tag="ps")
                for v in range(3):
                    nc.tensor.matmul(
                        ps[:rS, :cl],
                        self.bb[:rE, :rS],
                        U[:rE, cs + v : cs + v + cl],
                        start=(v == 0),
                        stop=(v == 2),
                    )
                nc.vector.tensor_mul(
                    out=res[:rS, cs : cs + cl],
                    in0=E[:rS, cs + 2 : cs + 2 + cl],
                    in1=ps[:rS, :cl],
                )
            if self.kind == "n":
                nc.sync.dma_start(
                    out=out_d[o : o + R, g0 : g0 + WH], in_=res[2 : R + 2, :WH]
                )
            else:
                for b in range(4):
                    nc.sync.dma_start(
                        out=out_d[o : o + R, g0 + b * SEGW : g0 + (b + 1) * SEGW],
                        in_=res[32 * b + 2 : 32 * b + 2 + R, :SEGW],
                    )

    of, Rf = tiles[-1]
    units = []
    for o, R in tiles[:-1]:
        for h in range(WS):
            units.append(Unit("n", o, R, h))
    units.insert(1, Unit("f", of, Rf, 0))
    units.append(Unit("f", of, Rf, WS - 1))

    LAG2, LAG3 = 1, 2
    n = len(units)
    for i in range(n + LAG3):
        if i < n:
            units[i].p1()
        if 0 <= i - LAG2 < n:
            units[i - LAG2].p2()
        if 0 <= i - LAG3 < n:
            units[i - LAG3].p3()


_CACHE: dict = {}


def _build():
    if "nc" in _CACHE:
        return _CACHE["nc"]
    nc = bacc.Bacc(
        "TRN2", target_bir_lowering=False, debug=False, num_devices=N_CORES
    )
    xh_d = nc.dram_tensor(
        "xh", (RC + 2 * HALO + 2, W + 2 * HALO), F32, kind="ExternalInput"
    ).ap()
    mask_d = nc.dram_tensor("mask", (RC + 2 * HALO, 1), F32, kind="ExternalInput").ap()
    maskf_d = nc.dram_tensor("maskf", (128, 1), F32, kind="ExternalInput").ap()
    bands_d = nc.dram_tensor("bands", (10, 128, 128), F32, kind="ExternalInput").ap()
    out_d = nc.dram_tensor("out", (RC, W), F32, kind="ExternalOutput").ap()
    with tile.TileContext(nc) as tc:
        _energy_body(tc, out_d, xh_d, mask_d, maskf_d, bands_d)
    nc.compile()
    _CACHE["nc"] = nc
    return nc


def kernel(shareable_energy: np.ndarray, kernel: np.ndarray, **_run_kw) -> np.ndarray:
    x = np.ascontiguousarray(np.asarray(shareable_energy, np.float32))
    k = np.asarray(kernel, np.float32)
    assert x.shape == (H, W), x.shape
    nc = _build()
    bands = _make_bands(k)
    in_maps = [_make_core_inputs(x, bands, core) for core in range(N_CORES)]
    r = run_bass_kernel_spmd(nc, in_maps, core_ids=list(range(N_CORES)), **_run_kw)
    out = np.concatenate([res["out"] for res in r.results], axis=0)
    if _run_kw:
        _CACHE["last_result"] = r
    return out

